# revision 6
# baseline (speedup 1.0000x reference)
"""Trainium2 Bass kernel for ChannelPredictor (dense_mlp).

Math (per batch sample, N=t*h*w=1024 tokens, D=NV=512, NC=4 channels):
  x = LayerNorm_d(yl)                       # (N, D), token-major in ref
  for k in 0..3:
    u_k = x @ Uk[:, :D].T + sum_{j<k} Uk[:, D+j*NV + idx_j].T + Uk_b
    o_k = relu(u_k) @ P.T + P_b

Reformulations used here:
  * one-hot @ Uk_w == gather of Uk_w columns -> dma_gather of bf16 embedding
    rows (tables pre-transposed host-side to (vocab, D) row-major).
  * the x-parts of all four U_k share rhs x -> one fused (D -> 4D) matmul.
  * LayerNorm folded: stats via ones-vector matmuls (fp32r), per-token
    mu/rstd broadcast across partitions via K=1 matmuls.

Sharding: data-parallel over batch b: 16 samples / 8 cores = 2 samples
(2048 tokens) per core; every core holds the full weights.
"""

import sys

for _p in ("/opt/trn_rl_repo",):
    if _p not in sys.path:
        sys.path.insert(0, _p)

import numpy as np
import ml_dtypes
from contextlib import ExitStack

import concourse.bass as bass
import concourse.bacc as bacc
import concourse.mybir as mybir
import concourse.tile as tile
from concourse import bass_utils

F32 = mybir.dt.float32
F32R = mybir.dt.float32r
BF16 = mybir.dt.bfloat16
I16 = mybir.dt.int16
AF = mybir.ActivationFunctionType
ALU = mybir.AluOpType
BF16NP = ml_dtypes.bfloat16

NCORES = 8
B, D, NV, NCH = 16, 512, 512, 4
T, H, W = 4, 16, 16
NTOK = T * H * W          # tokens per sample
BLOC = B // NCORES        # samples per core
TOK = BLOC * NTOK         # tokens per core
CH = 512                  # token chunk (matmul moving free dim)
NCHUNK = TOK // CH
EPS = 1e-5
PAIRS = [(1, 0), (2, 0), (2, 1), (3, 0), (3, 1), (3, 2)]  # (k, j) with j < k


def build_program():
    nc = bacc.Bacc("TRN2", target_bir_lowering=False, debug=False)

    x_d = nc.dram_tensor("x", (D, TOK), F32, kind="ExternalInput")
    wu_d = nc.dram_tensor("wu", (4, 128, 4 * D), BF16, kind="ExternalInput")
    wp_d = nc.dram_tensor("wp", (4, 128, NV), BF16, kind="ExternalInput")
    tab_d = {
        (k, j): nc.dram_tensor(f"tab{k}{j}", (NV, D), BF16, kind="ExternalInput")
        for (k, j) in PAIRS
    }
    idx_d = nc.dram_tensor("idx", (3, 128, TOK // 16), I16, kind="ExternalInput")
    ub_d = nc.dram_tensor("ub", (128, 16), F32, kind="ExternalInput")
    pb_d = nc.dram_tensor("pb", (128, 4), F32, kind="ExternalInput")
    gb_d = nc.dram_tensor("gb", (128, 8), F32, kind="ExternalInput")
    out_d = [
        nc.dram_tensor(f"out{k}", (4, NCHUNK, 128, CH), F32, kind="ExternalOutput")
        for k in range(NCH)
    ]

    with tile.TileContext(nc) as tc, ExitStack() as ctx:
        const = ctx.enter_context(tc.tile_pool(name="const", bufs=1))

        ones_k = const.tile([128, 1], BF16, name="ones_k")
        nc.vector.memset(ones_k[:], 1.0)
        ones_m = const.tile([1, 128], F32, name="ones_m")
        nc.vector.memset(ones_m[:], 1.0)
        eps_sb = const.tile([1, 1], F32, name="eps_sb")
        nc.vector.memset(eps_sb[:], EPS)

        wu_sb, wp_sb, x_sb, xn_sb = [], [], [], []
        for kt in range(4):
            w = const.tile([128, 4 * D], BF16, name=f"wu{kt}")
            nc.sync.dma_start(w[:], wu_d.ap()[kt])
            wu_sb.append(w)
            p = const.tile([128, NV], BF16, name=f"wp{kt}")
            nc.sync.dma_start(p[:], wp_d.ap()[kt])
            wp_sb.append(p)
            xt = const.tile([128, TOK], F32, name=f"x{kt}")
            nc.sync.dma_start(xt[:], x_d.ap()[kt * 128:(kt + 1) * 128, :])
            x_sb.append(xt)
            xn = const.tile([128, TOK], BF16, name=f"xn{kt}")
            xn_sb.append(xn)

        idx_sb = []
        for j in range(3):
            it = const.tile([128, TOK // 16], I16, name=f"idx{j}")
            nc.sync.dma_start(it[:], idx_d.ap()[j])
            idx_sb.append(it)

        ub_sb = const.tile([128, 16], F32, name="ub_sb")
        nc.sync.dma_start(ub_sb[:], ub_d.ap()[:, :])
        pb_sb = const.tile([128, 4], F32, name="pb_sb")
        nc.sync.dma_start(pb_sb[:], pb_d.ap()[:, :])
        gb_sb = const.tile([128, 8], F32, name="gb_sb")
        nc.sync.dma_start(gb_sb[:], gb_d.ap()[:, :])

        # ---------------- LayerNorm ----------------
        # Cross-partition stats via ones-vector matmuls (bf16 data, f32
        # accumulate); per-token mu/rstd broadcast to 128 partitions via
        # K=1 fp32 matmuls (fp32 keeps rstd full precision).
        xb_sb = []
        for kt in range(4):
            xb = const.tile([128, TOK], BF16, name=f"xb{kt}")
            nc.scalar.copy(xb[:], x_sb[kt][:])
            xb_sb.append(xb)

        lnp = ctx.enter_context(tc.tile_pool(name="lnp", bufs=1))
        with tc.tile_pool(name="ln_psum", bufs=1, space="PSUM") as lps:
            for c in range(NCHUNK):
                cs = bass.ts(c, CH)
                ps = lps.tile([1, CH], F32, tag="s", name=f"ps_{c}")
                pq = lps.tile([1, CH], F32, tag="q", name=f"pq_{c}")
                x2s = []
                for kt in range(4):
                    x2 = lnp.tile([128, CH], BF16, tag=f"x2_{kt}", bufs=2,
                                  name=f"x2_{c}_{kt}")
                    nc.vector.tensor_mul(x2[:], xb_sb[kt][:, cs], xb_sb[kt][:, cs])
                    x2s.append(x2)
                for kt in range(4):
                    nc.tensor.matmul(ps[:], ones_k[:],
                                     xb_sb[kt][:, cs],
                                     start=kt == 0, stop=kt == 3)
                for kt in range(4):
                    nc.tensor.matmul(pq[:], ones_k[:], x2s[kt][:],
                                     start=kt == 0, stop=kt == 3)
                mu = lnp.tile([1, CH], F32, tag="mu", bufs=2, name=f"mu_{c}")
                nc.scalar.mul(mu[:], ps[:], 1.0 / D)
                m2 = lnp.tile([1, CH], F32, tag="m2", bufs=2, name=f"m2_{c}")
                nc.scalar.mul(m2[:], pq[:], 1.0 / D)
                var = lnp.tile([1, CH], F32, tag="var", bufs=2, name=f"var_{c}")
                nc.vector.tensor_mul(var[:], mu[:], mu[:])
                nc.vector.tensor_sub(var[:], m2[:], var[:])
                sd = lnp.tile([1, CH], F32, tag="sd", bufs=2, name=f"sd_{c}")
                nc.scalar.activation(sd[:], var[:], AF.Sqrt, bias=eps_sb[:])
                rstd = lnp.tile([1, CH], F32, tag="rstd", bufs=2, name=f"rstd_{c}")
                nc.vector.reciprocal(rstd[:], sd[:])

                pmu = lps.tile([128, CH], F32, tag="bmu", name=f"pmu_{c}")
                nc.tensor.matmul(pmu[:], ones_m[:], mu[:], start=True, stop=True)
                prs = lps.tile([128, CH], F32, tag="brs", name=f"prs_{c}")
                nc.tensor.matmul(prs[:], ones_m[:], rstd[:], start=True, stop=True)

                for kt in range(4):
                    t1 = lnp.tile([128, CH], F32, tag="t1", bufs=3,
                                  name=f"t1_{c}_{kt}")
                    nc.vector.tensor_sub(t1[:], x_sb[kt][:, cs], pmu[:])
                    t2 = lnp.tile([128, CH], BF16, tag="t2", bufs=3,
                                  name=f"t2_{c}_{kt}")
                    nc.vector.tensor_mul(t2[:], t1[:], prs[:])
                    nc.vector.tensor_scalar(
                        xn_sb[kt][:, cs], t2[:],
                        gb_sb[:, kt:kt + 1], gb_sb[:, 4 + kt:4 + kt + 1],
                        ALU.mult, ALU.add)

        # ---------------- main: U matmul + gather-add + relu + P matmul ---
        gpool = ctx.enter_context(tc.tile_pool(name="gpool", bufs=2))
        upool = ctx.enter_context(tc.tile_pool(name="upool", bufs=2))
        apool = ctx.enter_context(tc.tile_pool(name="apool", bufs=3))
        opool = ctx.enter_context(tc.tile_pool(name="opool", bufs=4))
        mpsum = ctx.enter_context(tc.tile_pool(name="mpsum", bufs=4, space="PSUM"))

        for c in range(NCHUNK):
            cs = bass.ts(c, CH)
            g = {}
            for (k, j) in PAIRS:
                gt = gpool.tile([128, 4, CH], BF16, tag=f"g{k}{j}",
                                name=f"g{k}{j}_{c}")
                nc.gpsimd.dma_gather(
                    out_ap=gt[:],
                    in_ap=tab_d[(k, j)].ap(),
                    idxs_ap=idx_sb[j][:, c * (CH // 16):(c + 1) * (CH // 16)],
                    num_idxs=CH,
                    num_idxs_reg=CH,
                    elem_size=D,
                    transpose=True,
                )
                g[(k, j)] = gt

            for k in range(NCH):
                ur = []
                for cc in range(4):
                    mt = k * 4 + cc
                    py = mpsum.tile([128, CH], F32, tag="y", name=f"py_{c}_{mt}")
                    for kt in range(4):
                        nc.tensor.matmul(
                            py[:],
                            wu_sb[kt][:, mt * 128:(mt + 1) * 128],
                            xn_sb[kt][:, cs],
                            start=kt == 0, stop=kt == 3)
                    urt = upool.tile([128, CH], BF16, tag=f"ur{cc}",
                                     name=f"ur_{c}_{mt}")
                    if k == 0:
                        nc.scalar.activation(urt[:], py[:], AF.Relu,
                                             bias=ub_sb[:, mt:mt + 1])
                    else:
                        acc = py[:]
                        for jj in range(k):
                            nxt = apool.tile([128, CH], F32, tag="acc",
                                             name=f"acc_{c}_{mt}_{jj}")
                            nc.vector.tensor_add(nxt[:], acc,
                                                 g[(k, jj)][:, cc, :])
                            acc = nxt[:]
                        nc.scalar.activation(urt[:], acc, AF.Relu,
                                             bias=ub_sb[:, mt:mt + 1])
                    ur.append(urt)

                for mt2 in range(4):
                    po = mpsum.tile([128, CH], F32, tag="o", name=f"po_{c}_{k}_{mt2}")
                    for kt2 in range(4):
                        nc.tensor.matmul(
                            po[:],
                            wp_sb[kt2][:, mt2 * 128:(mt2 + 1) * 128],
                            ur[kt2][:],
                            start=kt2 == 0, stop=kt2 == 3)
                    ot = opool.tile([128, CH], F32, tag="ot", name=f"ot_{c}_{k}_{mt2}")
                    nc.scalar.activation(ot[:], po[:], AF.Identity,
                                         bias=pb_sb[:, mt2:mt2 + 1])
                    nc.sync.dma_start(out_d[k].ap()[mt2, c], ot[:])

    nc.compile()
    return nc


def make_in_maps(inputs):
    yl = np.ascontiguousarray(np.asarray(inputs["yl"], np.float32))
    slice_idx = np.asarray(inputs["slice_idx"]).astype(np.int64)
    Uw = [np.asarray(inputs[f"U{k}_w"], np.float32) for k in range(4)]
    Ub = [np.asarray(inputs[f"U{k}_b"], np.float32) for k in range(4)]
    Pw = np.asarray(inputs["P_w"], np.float32)
    Pb = np.asarray(inputs["P_b"], np.float32)
    gamma = np.asarray(inputs["ln_gamma"], np.float32)
    beta = np.asarray(inputs["ln_beta"], np.float32)

    wu = np.ascontiguousarray(
        np.concatenate([Uw[k][:, :D] for k in range(4)], axis=0).T
    ).reshape(4, 128, 4 * D).astype(BF16NP)
    wp = np.ascontiguousarray(Pw.T).reshape(4, 128, NV).astype(BF16NP)
    tabs = {
        (k, j): np.ascontiguousarray(
            Uw[k][:, D + j * NV: D + (j + 1) * NV].T
        ).astype(BF16NP)
        for (k, j) in PAIRS
    }
    ub = np.stack(Ub).reshape(4, 4, 128).transpose(2, 0, 1).reshape(128, 16)
    ub = np.ascontiguousarray(ub, np.float32)
    pb = np.ascontiguousarray(Pb.reshape(4, 128).T, np.float32)
    gb = np.ascontiguousarray(
        np.concatenate([gamma.reshape(4, 128).T, beta.reshape(4, 128).T], axis=1),
        np.float32)

    shared = {"wu": wu, "wp": wp, "ub": ub, "pb": pb, "gb": gb}
    for (k, j), t in tabs.items():
        shared[f"tab{k}{j}"] = t

    in_maps = []
    for core in range(NCORES):
        s0 = core * BLOC
        ylc = yl[s0:s0 + BLOC].reshape(BLOC, D, NTOK)
        x = np.ascontiguousarray(np.concatenate(list(ylc), axis=1))  # (D, TOK)
        idxc = slice_idx[s0:s0 + BLOC].reshape(BLOC, NCH, NTOK)
        idx_tok = np.concatenate([idxc[s] for s in range(BLOC)], axis=1)  # (4, TOK)
        wrapped = idx_tok[:3].reshape(3, TOK // 16, 16).transpose(0, 2, 1)  # (3,16,TOK//16)
        wrapped = np.ascontiguousarray(
            np.tile(wrapped, (1, 8, 1)).astype(np.int16))  # (3,128,TOK//16)
        in_maps.append({"x": x, "idx": wrapped, **shared})
    return in_maps


def assemble_outputs(results):
    outs = []
    for k in range(NCH):
        per_core = []
        for core in range(NCORES):
            o = results[core][f"out{k}"]  # (4, NCHUNK, 128, CH)
            o = o.transpose(0, 2, 1, 3).reshape(NV, TOK)  # (512, 2048)
            o = o.reshape(NV, BLOC, NTOK).transpose(1, 0, 2)
            per_core.append(o.reshape(BLOC, NV, T, H, W))
        outs.append(np.ascontiguousarray(np.concatenate(per_core, axis=0),
                                         np.float32))
    return tuple(outs)


_CACHE = {}


def kernel(**inputs):
    if "nc" not in _CACHE:
        _CACHE["nc"] = build_program()
    nc = _CACHE["nc"]
    in_maps = make_in_maps(inputs)
    res = bass_utils.run_bass_kernel_spmd(nc, in_maps, core_ids=list(range(NCORES)))
    return assemble_outputs(res.results)


# revision 7
# speedup vs baseline: 1.1899x; 1.1899x over previous
"""Trainium2 Bass kernel for ChannelPredictor (dense_mlp).

Math (per batch sample, N=t*h*w=1024 tokens, D=NV=512, NC=4 channels):
  x = LayerNorm_d(yl)                       # (N, D), token-major in ref
  for k in 0..3:
    u_k = x @ Uk[:, :D].T + sum_{j<k} Uk[:, D+j*NV + idx_j].T + Uk_b
    o_k = relu(u_k) @ P.T + P_b

Reformulations used here:
  * one-hot @ Uk_w == gather of Uk_w columns -> dma_gather of bf16 embedding
    rows (tables pre-transposed host-side to (vocab, D) row-major), written
    d-major across partitions (transpose=True) so they add directly onto the
    matmul PSUM tiles.
  * the x-parts of all four U_k share rhs x -> one fused (D -> 4D) matmul.
  * LayerNorm stats via ones-vector matmuls (bf16 data, f32 accumulate);
    per-token mu/rstd broadcast across partitions via K=1 fp32 matmuls.

Sharding: data-parallel over batch b: 16 samples / 8 cores = 2 samples
(2048 tokens) per core; every core holds the full weights.

Schedule: all gathers are emitted first (GPSIMD/SWDGE runs them during the
LayerNorm lead-in); LayerNorm and the U/P matmul stages are interleaved per
512-token chunk so the PE pipeline fills early and stays warm.
"""

import sys

for _p in ("/opt/trn_rl_repo",):
    if _p not in sys.path:
        sys.path.insert(0, _p)

import numpy as np
import ml_dtypes
from contextlib import ExitStack

import concourse.bass as bass
import concourse.bacc as bacc
import concourse.mybir as mybir
import concourse.tile as tile
from concourse import bass_utils

F32 = mybir.dt.float32
BF16 = mybir.dt.bfloat16
I16 = mybir.dt.int16
AF = mybir.ActivationFunctionType
ALU = mybir.AluOpType
BF16NP = ml_dtypes.bfloat16

NCORES = 8
B, D, NV, NCH = 16, 512, 512, 4
T, H, W = 4, 16, 16
NTOK = T * H * W          # tokens per sample
BLOC = B // NCORES        # samples per core
TOK = BLOC * NTOK         # tokens per core
CH = 512                  # token chunk (matmul moving free dim)
NCHUNK = TOK // CH
EPS = 1e-5
PAIRS = [(1, 0), (2, 0), (2, 1), (3, 0), (3, 1), (3, 2)]  # (k, j) with j < k


def build_program():
    nc = bacc.Bacc("TRN2", target_bir_lowering=False, debug=False)

    x_d = nc.dram_tensor("x", (4, NCHUNK, 128, CH), F32, kind="ExternalInput")
    wu_d = nc.dram_tensor("wu", (4, 128, 4 * D), BF16, kind="ExternalInput")
    wp_d = nc.dram_tensor("wp", (4, 128, NV), BF16, kind="ExternalInput")
    tab_d = {
        (k, j): nc.dram_tensor(f"tab{k}{j}", (NV, D), BF16, kind="ExternalInput")
        for (k, j) in PAIRS
    }
    idx_d = nc.dram_tensor("idx", (3, 128, TOK // 16), I16, kind="ExternalInput")
    ub_d = nc.dram_tensor("ub", (128, 16), F32, kind="ExternalInput")
    pb_d = nc.dram_tensor("pb", (128, 4), F32, kind="ExternalInput")
    gb_d = nc.dram_tensor("gb", (128, 8), F32, kind="ExternalInput")
    out_d = [
        nc.dram_tensor(f"out{k}", (4, NCHUNK, 128, CH), F32, kind="ExternalOutput")
        for k in range(NCH)
    ]

    with tile.TileContext(nc) as tc, ExitStack() as ctx:
        const = ctx.enter_context(tc.tile_pool(name="const", bufs=1))

        idx_sb = []
        for j in range(3):
            it = const.tile([128, TOK // 16], I16, name=f"idx{j}")
            nc.sync.dma_start(it[:], idx_d.ap()[j])
            idx_sb.append(it)

        ones_k = const.tile([128, 1], BF16, name="ones_k")
        nc.vector.memset(ones_k[:], 1.0)
        ones_m = const.tile([1, 128], F32, name="ones_m")
        nc.vector.memset(ones_m[:], 1.0)
        eps_sb = const.tile([1, 1], F32, name="eps_sb")
        nc.vector.memset(eps_sb[:], EPS)

        ub_sb = const.tile([128, 16], F32, name="ub_sb")
        nc.sync.dma_start(ub_sb[:], ub_d.ap()[:, :])
        pb_sb = const.tile([128, 4], F32, name="pb_sb")
        nc.sync.dma_start(pb_sb[:], pb_d.ap()[:, :])
        gb_sb = const.tile([128, 8], F32, name="gb_sb")
        nc.sync.dma_start(gb_sb[:], gb_d.ap()[:, :])

        # ---- gathers first: GPSIMD/SWDGE fills embedding tiles while the
        # rest of the pipeline boots.
        gpool = ctx.enter_context(tc.tile_pool(name="gpool", bufs=2))
        g = {}
        for c in range(NCHUNK):
            for (k, j) in PAIRS:
                gt = gpool.tile([128, 4, CH], BF16, tag=f"g{k}{j}",
                                name=f"g{k}{j}_{c}")
                nc.gpsimd.dma_gather(
                    out_ap=gt[:],
                    in_ap=tab_d[(k, j)].ap(),
                    idxs_ap=idx_sb[j][:, c * (CH // 16):(c + 1) * (CH // 16)],
                    num_idxs=CH,
                    num_idxs_reg=CH,
                    elem_size=D,
                    transpose=True,
                )
                g[(c, k, j)] = gt

        # ---- weights
        wu_sb, wp_sb = [], []
        for kt in range(4):
            w = const.tile([128, 4 * D], BF16, name=f"wu{kt}")
            nc.sync.dma_start(w[:], wu_d.ap()[kt])
            wu_sb.append(w)
            p = const.tile([128, NV], BF16, name=f"wp{kt}")
            nc.sync.dma_start(p[:], wp_d.ap()[kt])
            wp_sb.append(p)

        lnp = ctx.enter_context(tc.tile_pool(name="lnp", bufs=3))
        xpool = ctx.enter_context(tc.tile_pool(name="xpool", bufs=2))
        upool = ctx.enter_context(tc.tile_pool(name="upool", bufs=2))
        apool = ctx.enter_context(tc.tile_pool(name="apool", bufs=3))
        opool = ctx.enter_context(tc.tile_pool(name="opool", bufs=4))
        lps = ctx.enter_context(tc.tile_pool(name="lps", bufs=1, space="PSUM"))
        mps = ctx.enter_context(tc.tile_pool(name="mps", bufs=4, space="PSUM"))

        for c in range(NCHUNK):
            # ---------------- LayerNorm for this chunk ----------------
            xc, xbc = [], []
            for kt in range(4):
                xt = xpool.tile([128, CH], F32, tag=f"x{kt}", name=f"x_{c}_{kt}")
                nc.sync.dma_start(xt[:], x_d.ap()[kt, c])
                xc.append(xt)
                xb = xpool.tile([128, CH], BF16, tag=f"xb{kt}", name=f"xb_{c}_{kt}")
                nc.scalar.copy(xb[:], xt[:])
                xbc.append(xb)

            ps = lps.tile([1, CH], F32, tag="s", name=f"ps_{c}")
            pq = lps.tile([1, CH], F32, tag="q", name=f"pq_{c}")
            x2s = []
            for kt in range(4):
                x2 = lnp.tile([128, CH], BF16, tag=f"x2_{kt}", bufs=2,
                              name=f"x2_{c}_{kt}")
                nc.vector.tensor_mul(x2[:], xbc[kt][:], xbc[kt][:])
                x2s.append(x2)
            for kt in range(4):
                nc.tensor.matmul(ps[:], ones_k[:], xbc[kt][:],
                                 start=kt == 0, stop=kt == 3)
            for kt in range(4):
                nc.tensor.matmul(pq[:], ones_k[:], x2s[kt][:],
                                 start=kt == 0, stop=kt == 3)

            mu = lnp.tile([1, CH], F32, tag="mu", name=f"mu_{c}")
            nc.scalar.mul(mu[:], ps[:], 1.0 / D)
            m2 = lnp.tile([1, CH], F32, tag="m2", name=f"m2_{c}")
            nc.scalar.mul(m2[:], pq[:], 1.0 / D)
            var = lnp.tile([1, CH], F32, tag="var", name=f"var_{c}")
            nc.vector.tensor_mul(var[:], mu[:], mu[:])
            nc.vector.tensor_sub(var[:], m2[:], var[:])
            sd = lnp.tile([1, CH], F32, tag="sd", name=f"sd_{c}")
            nc.scalar.activation(sd[:], var[:], AF.Sqrt, bias=eps_sb[:])
            rstd = lnp.tile([1, CH], F32, tag="rstd", name=f"rstd_{c}")
            nc.vector.reciprocal_approx_fast(rstd[:], sd[:])

            pmu = lps.tile([128, CH], F32, tag="bmu", name=f"pmu_{c}")
            nc.tensor.matmul(pmu[:], ones_m[:], mu[:], start=True, stop=True)
            prs = lps.tile([128, CH], F32, tag="brs", name=f"prs_{c}")
            nc.tensor.matmul(prs[:], ones_m[:], rstd[:], start=True, stop=True)

            xnc = []
            for kt in range(4):
                t1 = lnp.tile([128, CH], F32, tag="t1", name=f"t1_{c}_{kt}")
                nc.vector.tensor_sub(t1[:], xc[kt][:], pmu[:])
                t2 = lnp.tile([128, CH], BF16, tag="t2", name=f"t2_{c}_{kt}")
                nc.vector.tensor_mul(t2[:], t1[:], prs[:])
                xn = xpool.tile([128, CH], BF16, tag=f"xn{kt}", name=f"xn_{c}_{kt}")
                nc.vector.tensor_scalar(
                    xn[:], t2[:],
                    gb_sb[:, kt:kt + 1], gb_sb[:, 4 + kt:4 + kt + 1],
                    ALU.mult, ALU.add)
                xnc.append(xn)

            # ---------------- U matmul + gather-add + relu + P matmul -----
            for k in range(NCH):
                ur = []
                for cc in range(4):
                    mt = k * 4 + cc
                    py = mps.tile([128, CH], F32, tag="ps", name=f"py_{c}_{mt}")
                    for kt in range(4):
                        nc.tensor.matmul(
                            py[:],
                            wu_sb[kt][:, mt * 128:(mt + 1) * 128],
                            xnc[kt][:],
                            start=kt == 0, stop=kt == 3)
                    urt = upool.tile([128, CH], BF16, tag=f"ur{cc}",
                                     name=f"ur_{c}_{mt}")
                    if k == 0:
                        nc.scalar.activation(urt[:], py[:], AF.Relu,
                                             bias=ub_sb[:, mt:mt + 1])
                    else:
                        acc = py[:]
                        for jj in range(k):
                            nxt = apool.tile([128, CH], F32, tag="acc",
                                             name=f"acc_{c}_{mt}_{jj}")
                            nc.vector.tensor_add(nxt[:], acc,
                                                 g[(c, k, jj)][:, cc, :])
                            acc = nxt[:]
                        nc.scalar.activation(urt[:], acc, AF.Relu,
                                             bias=ub_sb[:, mt:mt + 1])
                    ur.append(urt)

                for mt2 in range(4):
                    po = mps.tile([128, CH], F32, tag="ps", name=f"po_{c}_{k}_{mt2}")
                    for kt2 in range(4):
                        nc.tensor.matmul(
                            po[:],
                            wp_sb[kt2][:, mt2 * 128:(mt2 + 1) * 128],
                            ur[kt2][:],
                            start=kt2 == 0, stop=kt2 == 3)
                    ot = opool.tile([128, CH], F32, tag="ot", name=f"ot_{c}_{k}_{mt2}")
                    nc.scalar.activation(ot[:], po[:], AF.Identity,
                                         bias=pb_sb[:, mt2:mt2 + 1])
                    nc.sync.dma_start(out_d[k].ap()[mt2, c], ot[:])

    nc.compile()
    return nc


def make_in_maps(inputs):
    yl = np.ascontiguousarray(np.asarray(inputs["yl"], np.float32))
    slice_idx = np.asarray(inputs["slice_idx"]).astype(np.int64)
    Uw = [np.asarray(inputs[f"U{k}_w"], np.float32) for k in range(4)]
    Ub = [np.asarray(inputs[f"U{k}_b"], np.float32) for k in range(4)]
    Pw = np.asarray(inputs["P_w"], np.float32)
    Pb = np.asarray(inputs["P_b"], np.float32)
    gamma = np.asarray(inputs["ln_gamma"], np.float32)
    beta = np.asarray(inputs["ln_beta"], np.float32)

    wu = np.ascontiguousarray(
        np.concatenate([Uw[k][:, :D] for k in range(4)], axis=0).T
    ).reshape(4, 128, 4 * D).astype(BF16NP)
    wp = np.ascontiguousarray(Pw.T).reshape(4, 128, NV).astype(BF16NP)
    tabs = {
        (k, j): np.ascontiguousarray(
            Uw[k][:, D + j * NV: D + (j + 1) * NV].T
        ).astype(BF16NP)
        for (k, j) in PAIRS
    }
    ub = np.stack(Ub).reshape(4, 4, 128).transpose(2, 0, 1).reshape(128, 16)
    ub = np.ascontiguousarray(ub, np.float32)
    pb = np.ascontiguousarray(Pb.reshape(4, 128).T, np.float32)
    gb = np.ascontiguousarray(
        np.concatenate([gamma.reshape(4, 128).T, beta.reshape(4, 128).T], axis=1),
        np.float32)

    shared = {"wu": wu, "wp": wp, "ub": ub, "pb": pb, "gb": gb}
    for (k, j), t in tabs.items():
        shared[f"tab{k}{j}"] = t

    in_maps = []
    for core in range(NCORES):
        s0 = core * BLOC
        ylc = yl[s0:s0 + BLOC].reshape(BLOC, D, NTOK)
        x = np.concatenate(list(ylc), axis=1)            # (D, TOK)
        x = np.ascontiguousarray(
            x.reshape(4, 128, NCHUNK, CH).transpose(0, 2, 1, 3))
        idxc = slice_idx[s0:s0 + BLOC].reshape(BLOC, NCH, NTOK)
        idx_tok = np.concatenate([idxc[s] for s in range(BLOC)], axis=1)  # (4, TOK)
        wrapped = idx_tok[:3].reshape(3, TOK // 16, 16).transpose(0, 2, 1)
        wrapped = np.ascontiguousarray(
            np.tile(wrapped, (1, 8, 1)).astype(np.int16))  # (3,128,TOK//16)
        in_maps.append({"x": x, "idx": wrapped, **shared})
    return in_maps


def assemble_outputs(results):
    outs = []
    for k in range(NCH):
        per_core = []
        for core in range(NCORES):
            o = results[core][f"out{k}"]  # (4, NCHUNK, 128, CH)
            o = o.transpose(0, 2, 1, 3).reshape(NV, TOK)  # (512, 2048)
            o = o.reshape(NV, BLOC, NTOK).transpose(1, 0, 2)
            per_core.append(o.reshape(BLOC, NV, T, H, W))
        outs.append(np.ascontiguousarray(np.concatenate(per_core, axis=0),
                                         np.float32))
    return tuple(outs)


_CACHE = {}


def kernel(**inputs):
    if "nc" not in _CACHE:
        _CACHE["nc"] = build_program()
    nc = _CACHE["nc"]
    in_maps = make_in_maps(inputs)
    res = bass_utils.run_bass_kernel_spmd(nc, in_maps, core_ids=list(range(NCORES)))
    return assemble_outputs(res.results)
